# revision 25
# baseline (speedup 1.0000x reference)
"""BitFeedForward (BitNet-style FFN) Trainium2 kernel — 8-core data parallel.

kernel(**inputs) takes the FULL unsharded inputs of
nn_BitFeedForward_25280177504455:
    x  [4, 2048, 2048] f32, w1 [8192, 2048], b1 [8192],
    w2 [2048, 8192], b2 [2048]
and returns the full [4, 2048, 2048] f32 output.

Sharding: data-parallel over tokens (1024 tokens/core).  The host ships
weights pre-transposed (w1.T / w2.T, a pure layout change) so each core
can stream them with large contiguous DMA descriptors, convert to
ternary sign form on the fly (sign -> bf16 in SBUF, no DRAM spill), and
use them directly as matmul operands.  weight_quant's global mean|w| is
computed from per-core shards and combined with a tiny 8-core AllReduce
(8 bytes), so the full |w| reduction is done once across the chip
instead of 8x redundantly.

On-device flow per core (1024 tokens):
  A. |w| partial sums over this core's weight shard -> AllReduce -> mw1, mw2
  B. x stats (rms + absmax) -> r1/c1/beta1, quantize, PE-transpose -> q1T
     (SBUF resident)
  C. L1: stream w1T in 512-col slabs, sign to bf16, matmul
     h[t,i] = gelu(beta1*(q1T.T @ w1s + rb1 x b1)); bn_stats/absmax per
     token accumulate for the second rms; h spilled bf16.
  D. finalize r2/beta2; rebuild q2 from h, PE-transpose -> q2T (SBUF
     resident, aliases q1T's space)
  E. L2: stream w2T in 128-row o-bands, sign, matmul
     out[o,t] = beta2[t]*(w2s.T @ q2T + b2 x rb2) -> out written [o,t];
     host transposes back.
"""
import functools

import numpy as np
import ml_dtypes

from contextlib import ExitStack

import concourse.bacc as bacc
import concourse.tile as tile
from concourse import mybir
from concourse.bass_utils import run_bass_kernel_spmd

F32 = mybir.dt.float32
BF16 = mybir.dt.bfloat16

EPS_RMS = 1e-6
EPS_Q = 1e-5
# v + C lands in [2^23, 2^24) where fp32 spacing is 1.0 -> RNE integer round
C_RND = float(1.5 * 2.0**23)
P = 128
AX = mybir.AxisListType
ALU = mybir.AluOpType
AF = mybir.ActivationFunctionType

NCORES = 8
B, S, DIM = 4, 2048, 2048
INNER = 8192
OUT = DIM
TOK = B * S // NCORES   # 1024 tokens per core
TT = TOK // P           # 8 token tiles
KD = DIM // P           # 16 contraction chunks for L1
KI = INNER // P         # 64 contraction chunks for L2
NE1 = 16                # L1 slabs (512 inner cols each)
SL1 = INNER // NE1      # 512
NB2 = 16                # L2 o-bands (128 out cols each)
BO = OUT // NB2         # 128
WSHE = 2 * INNER * DIM // NCORES  # weight-shard elements per core (4.19M)
WSHC = WSHE // (P * 2048)         # 16 chunks of [128, 2048]


def build():
    from concourse.tile_rust import add_dep_helper

    nc = bacc.Bacc("TRN2", enable_partition_id=False, num_devices=NCORES)

    x_d = nc.dram_tensor("x", [TOK, DIM], F32, kind="ExternalInput")
    w1t_d = nc.dram_tensor("w1t", [DIM, INNER], F32, kind="ExternalInput")
    w2t_d = nc.dram_tensor("w2t", [INNER, OUT], F32, kind="ExternalInput")
    b1_d = nc.dram_tensor("b1", [1, INNER], F32, kind="ExternalInput")
    b2_d = nc.dram_tensor("b2", [1, OUT], F32, kind="ExternalInput")
    wsh_d = nc.dram_tensor("wsh", [P, WSHE // P], F32, kind="ExternalInput")
    idf_d = nc.dram_tensor("identf", [P, P], F32, kind="ExternalInput")
    idb_d = nc.dram_tensor("identb", [P, P], BF16, kind="ExternalInput")
    ones_d = nc.dram_tensor("ones", [P, P], F32, kind="ExternalInput")
    out_d = nc.dram_tensor("out", [OUT, TOK], F32, kind="ExternalOutput")

    with ExitStack() as ctx:
        tc = ctx.enter_context(tile.TileContext(nc))
        pool = lambda name, bufs, space="SBUF": ctx.enter_context(
            tc.tile_pool(name=name, bufs=bufs, space=space))

        consts = pool("consts", 1)
        stag = pool("stag", 3)        # f32 staging: w1T/w2T slab tiles
        stagx = pool("stagx", 2)      # f32 staging: x tiles
        wshp = pool("wshp", 2)        # f32 staging: wsh tiles
        wring = pool("wring", 2)      # bf16 sign-weight slabs
        qTp = pool("qTp", 1)          # q1T then q2T (aliased)
        qp = pool("qp", 1)            # q1 bf16 chunks
        hcp = pool("hcp", 2)          # h / q2 bf16 tiles
        outp = pool("outp", 2)        # f32 drains
        vecs = pool("vecs", 2)
        bch = pool("bch", 1)
        bb = pool("bb", 1)
        pers = pool("pers", 1)
        dram = pool("dram", 1, "DRAM")
        ps_g = pool("ps_g", 4, "PSUM")
        ps_t = pool("ps_t", 2, "PSUM")
        ps_v = pool("ps_v", 2, "PSUM")

        identf = consts.tile([P, P], F32)
        identb = consts.tile([P, P], BF16)
        ones = consts.tile([P, P], F32)
        nc.sync.dma_start(identf, idf_d[:, :])
        nc.sync.dma_start(identb, idb_d[:, :])
        nc.sync.dma_start(ones, ones_d[:, :])

        h_dram = dram.tile([TOK, INNER], BF16)
        cc_in = dram.tile([1, 2], F32)
        cc_out = dram.tile([1, 2], F32, addr_space="Shared")

        state = {"pe": None}

        def pe(instr):
            if state["pe"] is not None:
                add_dep_helper(instr.ins, state["pe"].ins, sync=False,
                               reason="pe chain")
            state["pe"] = instr
            return instr

        # ---- persistent scalars/vectors ----
        wsums4 = pers.tile([P, WSHC * 4], F32, tag="wsums4")
        beta1s = pers.tile([P, TT], F32, tag="beta1s")
        r2s = pers.tile([P, TT], F32, tag="r2s")
        m2s = pers.tile([P, TT], F32, tag="m2s")
        rb1T = pers.tile([1, TOK], F32, tag="rb1T")
        rb2row = pers.tile([1, TOK], F32, tag="rb2row")
        beta2row = pers.tile([1, TOK], F32, tag="beta2row")
        mws = pers.tile([P, 2], F32, tag="mws")
        stv2 = pers.tile([P, TT, NE1, 6], F32, tag="stv2")

        # ================= A: weight-shard |w| sums + AllReduce ========
        # |w| partial sums ride the scalar engine (Abs + accum_out) so the
        # vector engine is free for the concurrent x-statistics.
        for j in range(WSHC * 4):
            wt = wshp.tile([P, 512], F32, tag="wsh")
            nc.sync.dma_start(wt, wsh_d[:, j * 512:(j + 1) * 512])
            nc.scalar.activation(wt, wt, AF.Abs,
                                 accum_out=wsums4[:, j:j + 1])
        cc_sb = vecs.tile([1, 2], F32, tag="cc_sb")
        for half in range(2):
            col = vecs.tile([P, 1], F32, tag="wcol")
            nc.vector.tensor_reduce(
                col, wsums4[:, half * (WSHC * 2):(half + 1) * (WSHC * 2)],
                axis=AX.X, op=ALU.add)
            pssc = ps_v.tile([1, 1], F32, tag="psv")
            pe(nc.tensor.matmul(pssc, col, ones[:, 0:1], start=True,
                                stop=True))
            nc.scalar.copy(cc_sb[0:1, half:half + 1], pssc)
        nc.sync.dma_start(cc_in, cc_sb)
        nc.gpsimd.collective_compute(
            "AllReduce", ALU.add,
            replica_groups=[list(range(NCORES))],
            ins=[cc_in.opt()], outs=[cc_out.opt()])
        cc_rb = vecs.tile([1, 2], F32, tag="cc_rb")
        nc.sync.dma_start(cc_rb, cc_out)
        for half in range(2):
            psbc = ps_v.tile([P, 1], F32, tag="psv")
            pe(nc.tensor.matmul(psbc, ones[0:1, :], cc_rb[0:1, half:half + 1],
                                start=True, stop=True))
            nc.scalar.mul(mws[:, half:half + 1], psbc,
                          1.0 / (INNER * DIM * 127.0))
        mw1 = mws[:, 0:1]
        mw2 = mws[:, 1:2]

        def finalize_scale(stv, M, WID, r_out):
            mv = vecs.tile([P, 2], F32, tag="bn_mv")
            nc.vector.bn_aggr(mv, stv)
            msq = vecs.tile([P, 1], F32, tag="msq")
            nc.vector.tensor_tensor(msq, mv[:, 0:1], mv[:, 0:1], op=ALU.mult)
            nc.vector.tensor_tensor(msq, msq, mv[:, 1:2], op=ALU.add)
            nc.vector.tensor_scalar_add(msq, msq, EPS_RMS)
            y = vecs.tile([P, 1], F32, tag="sq_y")
            nc.scalar.sqrt(y, msq)
            d_ = vecs.tile([P, 1], F32, tag="sq_d")
            nc.vector.reciprocal(d_, y)
            nc.vector.tensor_tensor(d_, msq, d_, op=ALU.mult)
            nc.vector.tensor_tensor(y, y, d_, op=ALU.add)
            nc.vector.tensor_scalar_mul(y, y, 0.5 * (float(WID) ** 0.5))
            a = vecs.tile([P, 1], F32, tag="a")
            nc.vector.reciprocal(a, y)
            c = vecs.tile([P, 1], F32, tag="c")
            nc.vector.tensor_tensor(c, a, M, op=ALU.mult)
            nc.vector.tensor_scalar_max(c, c, EPS_Q)
            r = vecs.tile([P, 1], F32, tag="r")
            nc.vector.reciprocal(r, c)
            nc.vector.tensor_tensor(r, r, a, op=ALU.mult)
            nc.vector.tensor_scalar_mul(r_out, r, 127.0)
            return c

        def col_to_row(col, row_slice):
            pst = ps_v.tile([1, P], F32, tag="psv")
            pe(nc.tensor.transpose(pst, col, identf))
            nc.scalar.copy(row_slice, pst)

        # ================= B: x-phase ==================================
        q1T = qTp.tile([P, KD, TOK], BF16, tag="qT", name="q1T")
        for tt in range(TT):
            stv = vecs.tile([P, 4, 6], F32, tag="stv1")
            M1 = vecs.tile([P, 1], F32, tag="M1")
            for cc in range(4):
                xt = stagx.tile([P, 512], F32, tag="stagx")
                nc.sync.dma_start(xt, x_d[tt * P:(tt + 1) * P,
                                          cc * 512:(cc + 1) * 512])
                nc.vector.bn_stats(stv[:, cc, :], xt)
                mx = vecs.tile([P, 1], F32, tag="mx")
                nc.vector.tensor_reduce(mx, xt, axis=AX.X, op=ALU.max,
                                        apply_absolute_value=True)
                if cc == 0:
                    nc.vector.tensor_copy(out=M1, in_=mx)
                else:
                    nc.vector.tensor_tensor(M1, M1, mx, op=ALU.max)
            r1 = vecs.tile([P, 1], F32, tag="r1")
            c1 = finalize_scale(stv, M1, DIM, r1)
            beta1 = beta1s[:, tt:tt + 1]
            nc.vector.tensor_tensor(beta1, c1, mw1, op=ALU.mult)
            rb1 = vecs.tile([P, 1], F32, tag="rb1")
            nc.vector.reciprocal(rb1, beta1)
            col_to_row(rb1, rb1T[0:1, tt * P:(tt + 1) * P])
            for cc in range(4):
                xt = stagx.tile([P, 512], F32, tag="stagx")
                nc.sync.dma_start(xt, x_d[tt * P:(tt + 1) * P,
                                          cc * 512:(cc + 1) * 512])
                xq = stagx.tile([P, 512], F32, tag="stagx")
                nc.vector.tensor_scalar(xq, xt, r1, C_RND, op0=ALU.mult,
                                        op1=ALU.add)
                q1 = qp.tile([P, 512], BF16, tag="q1")
                nc.scalar.activation(q1, xq, AF.Copy, bias=-C_RND)
                pst = ps_t.tile([P, 512], BF16, tag="pst")
                for j in range(4):
                    pe(nc.tensor.transpose(pst[:, j * P:(j + 1) * P],
                                           q1[:, j * P:(j + 1) * P], identb))
                nc.vector.tensor_copy(
                    out=q1T[:, 4 * cc:4 * (cc + 1), tt * P:(tt + 1) * P],
                    in_=pst.rearrange("p (a b) -> p a b", b=P))

        # ================= C: L1 slabs =================================
        for e in range(NE1):
            ws = wring.tile([P, KD, SL1], BF16, tag="w", name=f"ws1_{e}")
            for dc in range(KD):
                wt = stag.tile([P, SL1], F32, tag="stag")
                nc.sync.dma_start(
                    wt, w1t_d[dc * P:(dc + 1) * P, e * SL1:(e + 1) * SL1])
                nc.scalar.sign(ws[:, dc, :], wt)
            bc = bch.tile([1, SL1], F32, tag="bc")
            nc.sync.dma_start(bc, b1_d[0:1, e * SL1:(e + 1) * SL1])
            for tt in range(TT):
                pg = ps_g.tile([P, SL1], F32, tag="psg")
                pe(nc.tensor.matmul(pg, rb1T[0:1, tt * P:(tt + 1) * P], bc,
                                    start=True, stop=False))
                for dc in range(KD):
                    pe(nc.tensor.matmul(pg, q1T[:, dc, tt * P:(tt + 1) * P],
                                        ws[:, dc, :], start=False,
                                        stop=(dc == KD - 1)))
                hc = hcp.tile([P, SL1], BF16, tag="h")
                nc.scalar.activation(hc, pg, AF.Gelu,
                                     scale=beta1s[:, tt:tt + 1])
                nc.vector.bn_stats(stv2[:, tt, e, :], hc)
                mx = vecs.tile([P, 1], F32, tag="mx")
                nc.vector.tensor_reduce(mx, hc, axis=AX.X, op=ALU.max,
                                        apply_absolute_value=True)
                m2 = m2s[:, tt:tt + 1]
                if e == 0:
                    nc.vector.tensor_copy(out=m2, in_=mx)
                else:
                    nc.vector.tensor_tensor(m2, m2, mx, op=ALU.max)
                nc.sync.dma_start(
                    h_dram[tt * P:(tt + 1) * P, e * SL1:(e + 1) * SL1], hc)

        # ================= finalize L2 scales ==========================
        for tt in range(TT):
            r2 = r2s[:, tt:tt + 1]
            c2 = finalize_scale(stv2[:, tt, :, :], m2s[:, tt:tt + 1], INNER,
                                r2)
            beta2 = vecs.tile([P, 1], F32, tag="beta2")
            nc.vector.tensor_tensor(beta2, c2, mw2, op=ALU.mult)
            rb2 = vecs.tile([P, 1], F32, tag="rb2")
            nc.vector.reciprocal(rb2, beta2)
            col_to_row(rb2, rb2row[0:1, tt * P:(tt + 1) * P])
            col_to_row(beta2, beta2row[0:1, tt * P:(tt + 1) * P])
        bb0 = bb.tile([P, 512], F32, tag="bb0")
        bb1 = bb.tile([P, 512], F32, tag="bb1")
        nc.gpsimd.partition_broadcast(bb0, beta2row[0:1, 0:512])
        nc.gpsimd.partition_broadcast(bb1, beta2row[0:1, 512:1024])
        bbs = [bb0, bb1]

        # ================= D: rebuild q2, transpose -> q2T =============
        q2T = qTp.tile([P, KI, TOK], BF16, tag="qT", name="q2T")
        for tt in range(TT):
            for ic in range(INNER // 512):
                hr = hcp.tile([P, 512], BF16, tag="h")
                nc.sync.dma_start(
                    hr, h_dram[tt * P:(tt + 1) * P, ic * 512:(ic + 1) * 512])
                hq = stagx.tile([P, 512], F32, tag="stagx")
                nc.vector.tensor_scalar(hq, hr, r2s[:, tt:tt + 1], C_RND,
                                        op0=ALU.mult, op1=ALU.add)
                q2c = hcp.tile([P, 512], BF16, tag="q2c")
                nc.scalar.activation(q2c, hq, AF.Copy, bias=-C_RND)
                pst = ps_t.tile([P, 512], BF16, tag="pst")
                for j in range(4):
                    pe(nc.tensor.transpose(pst[:, j * P:(j + 1) * P],
                                           q2c[:, j * P:(j + 1) * P], identb))
                nc.vector.tensor_copy(
                    out=q2T[:, 4 * ic:4 * (ic + 1), tt * P:(tt + 1) * P],
                    in_=pst.rearrange("p (a b) -> p a b", b=P))

        # ================= E: L2 o-bands ===============================
        for b in range(NB2):
            ws2 = wring.tile([P, KI, BO], BF16, tag="w", name=f"ws2_{b}")
            for g in range(16):
                wt = stag.tile([P, 4, BO], F32, tag="stag")
                nc.sync.dma_start(
                    wt,
                    w2t_d[g * 512:(g + 1) * 512,
                          b * BO:(b + 1) * BO].rearrange(
                              "(k p) o -> p k o", p=P))
                nc.scalar.sign(ws2[:, 4 * g:4 * (g + 1), :], wt)
            bc2 = bch.tile([1, BO], F32, tag="bc")
            nc.sync.dma_start(bc2, b2_d[0:1, b * BO:(b + 1) * BO])
            pbs = [ps_g.tile([P, 512], F32, tag="psg", name=f"pb{b}_{t}")
                   for t in range(2)]
            for tg in range(2):
                pe(nc.tensor.matmul(pbs[tg], bc2,
                                    rb2row[0:1, tg * 512:(tg + 1) * 512],
                                    start=True, stop=False))
            for kc in range(KI):
                for tg in range(2):
                    pe(nc.tensor.matmul(pbs[tg], ws2[:, kc, :],
                                        q2T[:, kc, tg * 512:(tg + 1) * 512],
                                        start=False, stop=(kc == KI - 1)))
            for tg in range(2):
                ob = outp.tile([P, 512], F32, tag="ob")
                nc.vector.tensor_tensor(ob, pbs[tg], bbs[tg], op=ALU.mult)
                nc.sync.dma_start(
                    out_d[b * BO:(b + 1) * BO, tg * 512:(tg + 1) * 512], ob)

    nc.compile()
    return nc


@functools.lru_cache(maxsize=1)
def _get_nc():
    return build()


def kernel(x, w1, b1, w2, b2, _trace=False):
    nc = _get_nc()
    xf = np.ascontiguousarray(x.reshape(B * S, DIM), dtype=np.float32)
    w1 = np.asarray(w1, dtype=np.float32)
    w2 = np.asarray(w2, dtype=np.float32)
    w1f = w1.reshape(-1)
    w2f = w2.reshape(-1)
    shard = w1f.size // NCORES
    common = {
        "w1t": np.ascontiguousarray(w1.T),
        "w2t": np.ascontiguousarray(w2.T),
        "b1": np.ascontiguousarray(b1, dtype=np.float32).reshape(1, INNER),
        "b2": np.ascontiguousarray(b2, dtype=np.float32).reshape(1, OUT),
        "identf": np.eye(P, dtype=np.float32),
        "identb": np.eye(P, dtype=np.float32).astype(ml_dtypes.bfloat16),
        "ones": np.ones((P, P), dtype=np.float32),
    }
    in_maps = []
    for c in range(NCORES):
        wsh = np.concatenate([
            w1f[c * shard:(c + 1) * shard].reshape(P, -1),
            w2f[c * shard:(c + 1) * shard].reshape(P, -1)], axis=1)
        in_maps.append({
            "x": xf[c * TOK:(c + 1) * TOK],
            "wsh": np.ascontiguousarray(wsh),
            **common,
        })
    res = run_bass_kernel_spmd(nc, in_maps, core_ids=list(range(NCORES)),
                               trace=_trace)
    out = np.concatenate(
        [res.results[c]["out"].T for c in range(NCORES)], axis=0)
    out = out.reshape(B, S, DIM)
    if _trace:
        return out, res
    return out


# revision 27
# speedup vs baseline: 1.2589x; 1.2589x over previous
"""BitFeedForward (BitNet-style FFN) Trainium2 kernel — 8-core data parallel.

kernel(**inputs) takes the FULL unsharded inputs of
nn_BitFeedForward_25280177504455:
    x  [4, 2048, 2048] f32, w1 [8192, 2048], b1 [8192],
    w2 [2048, 8192], b2 [2048]
and returns the full [4, 2048, 2048] f32 output.

Sharding: data-parallel over tokens (1024 tokens/core).  The host ships
weights pre-transposed (w1.T / w2.T, a pure layout change) so each core
streams them with large contiguous DMA descriptors, converts to ternary
sign form on the fly (sign -> bf16 in SBUF, no DRAM spill), and uses
them directly as matmul operands.  weight_quant's global mean|w| is
reduced from per-core shards with two tiny 8-core AllGathers (8 bytes),
issued early so their ~50us control-plane latency hides behind the
x-quantization phase.

Per-core flow:
  A. |w| partial sums over this core's weight shard (scalar engine),
     AllGather the two partials across cores.
  B. x stats -> batched scale finalize ([P,8] vector ops, one sqrt) ->
     quantize + PE-transpose -> q1T resident in SBUF.  The mean|w|-
     dependent beta1/rb1 math is emitted after this block so no engine
     queue stalls on the collectives.
  C. L1: stream w1T in 512-col slabs, sign to bf16, matmul
     h[t,i] = gelu(beta1*(q1T.T @ w1s + rb1 x b1)); per-token bn_stats/
     absmax accumulate; h spilled bf16.
  D. batched r2/beta2 finalize; rebuild q2 from h, PE-transpose -> q2T
     (aliases q1T's SBUF).
  E. L2: stream w2T in 128-row o-bands, sign, matmul both 512-token
     groups interleaved per stationary chunk:
     out[o,t] = beta2[t]*(w2s.T @ q2T + b2 x rb2), written bf16 [o,t];
     host transposes and upcasts.
"""
import functools

import numpy as np
import ml_dtypes

from contextlib import ExitStack

import concourse.bacc as bacc
import concourse.tile as tile
from concourse import mybir
from concourse.bass_utils import run_bass_kernel_spmd

F32 = mybir.dt.float32
BF16 = mybir.dt.bfloat16

EPS_RMS = 1e-6
EPS_Q = 1e-5
# v + C lands in [2^23, 2^24) where fp32 spacing is 1.0 -> RNE integer round
C_RND = float(1.5 * 2.0**23)
P = 128
AX = mybir.AxisListType
ALU = mybir.AluOpType
AF = mybir.ActivationFunctionType

NCORES = 8
B, S, DIM = 4, 2048, 2048
INNER = 8192
OUT = DIM
TOK = B * S // NCORES   # 1024 tokens per core
TT = TOK // P           # 8 token tiles
KD = DIM // P           # 16 contraction chunks for L1
KI = INNER // P         # 64 contraction chunks for L2
NE1 = 16                # L1 slabs (512 inner cols each)
SL1 = INNER // NE1      # 512
NB2 = 16                # L2 o-bands (128 out cols each)
BO = OUT // NB2         # 128
WSHE = 2 * INNER * DIM // NCORES  # weight-shard elements per core (4.19M)
WHALF = WSHE // (2 * P * 512)     # 512-chunks per w half (32)


def build():
    from concourse.tile_rust import add_dep_helper

    nc = bacc.Bacc("TRN2", enable_partition_id=False, num_devices=NCORES)

    x_d = nc.dram_tensor("x", [TOK, DIM], F32, kind="ExternalInput")
    w1t_d = nc.dram_tensor("w1t", [DIM, INNER], F32, kind="ExternalInput")
    w2t_d = nc.dram_tensor("w2t", [INNER, OUT], F32, kind="ExternalInput")
    b1_d = nc.dram_tensor("b1", [1, INNER], BF16, kind="ExternalInput")
    b2_d = nc.dram_tensor("b2", [1, OUT], BF16, kind="ExternalInput")
    wsh_d = nc.dram_tensor("wsh", [P, WSHE // P], F32, kind="ExternalInput")
    idf_d = nc.dram_tensor("identf", [P, P], F32, kind="ExternalInput")
    idb_d = nc.dram_tensor("identb", [P, P], BF16, kind="ExternalInput")
    ones_d = nc.dram_tensor("ones", [P, P], F32, kind="ExternalInput")
    out_d = nc.dram_tensor("out", [OUT, TOK], BF16, kind="ExternalOutput")

    with ExitStack() as ctx:
        tc = ctx.enter_context(tile.TileContext(nc))
        pool = lambda name, bufs, space="SBUF": ctx.enter_context(
            tc.tile_pool(name=name, bufs=bufs, space=space))

        consts = pool("consts", 1)
        stag = pool("stag", 2)        # f32 staging: w1T/w2T slab tiles
        stagx = pool("stagx", 3)      # f32 staging: x / h-requant tiles
        wshp = pool("wshp", 2)        # f32 staging: wsh tiles
        wring = pool("wring", 2)      # bf16 sign-weight slabs
        qTp = pool("qTp", 1)          # q1T then q2T (aliased)
        qp = pool("qp", 2)            # q1 bf16 chunks
        hcp = pool("hcp", 3)          # h / q2 bf16 tiles
        outp = pool("outp", 2)        # bf16 output drains
        vecs = pool("vecs", 4)
        bch = pool("bch", 1)
        bb = pool("bb", 1)
        pers = pool("pers", 1)
        dram = pool("dram", 1, "DRAM")
        ps_g = pool("ps_g", 4, "PSUM")
        ps_t = pool("ps_t", 2, "PSUM")
        ps_v = pool("ps_v", 2, "PSUM")

        identf = consts.tile([P, P], F32)
        identb = consts.tile([P, P], BF16)
        ones = consts.tile([P, P], F32)
        nc.sync.dma_start(identf, idf_d[:, :])
        nc.sync.dma_start(identb, idb_d[:, :])
        nc.sync.dma_start(ones, ones_d[:, :])

        h_dram = dram.tile([TOK, INNER], BF16)
        cc1_in = dram.tile([1, 1], F32)
        cc1_out = dram.tile([NCORES, 1], F32, addr_space="Shared")
        cc2_in = dram.tile([1, 1], F32)
        cc2_out = dram.tile([NCORES, 1], F32, addr_space="Shared")

        state = {"pe": None}

        def pe(instr):
            if state["pe"] is not None:
                add_dep_helper(instr.ins, state["pe"].ins, sync=False,
                               reason="pe chain")
            state["pe"] = instr
            return instr

        # ---- persistent scalars/vectors ----
        wsums = pers.tile([P, 2 * WHALF], F32, tag="wsums")
        stvx = pers.tile([P, TT, 4, 6], F32, tag="stvx")
        M1s = pers.tile([P, TT], F32, tag="M1s")
        r1s = pers.tile([P, TT], F32, tag="r1s")
        c1s = pers.tile([P, TT], F32, tag="c1s")
        beta1s = pers.tile([P, TT], F32, tag="beta1s")
        r2s = pers.tile([P, TT], F32, tag="r2s")
        m2s = pers.tile([P, TT], F32, tag="m2s")
        rb1T = pers.tile([1, TOK], BF16, tag="rb1T")
        rb2row = pers.tile([1, TOK], BF16, tag="rb2row")
        beta2row = pers.tile([1, TOK], F32, tag="beta2row")
        mws = pers.tile([P, 2], F32, tag="mws")
        stv2 = pers.tile([P, TT, NE1, 6], F32, tag="stv2")

        def wshard_sums(half):
            for j in range(WHALF):
                wt = wshp.tile([P, 512], F32, tag="wsh")
                nc.scalar.dma_start(
                    wt, wsh_d[:, (half * WHALF + j) * 512:
                              (half * WHALF + j + 1) * 512])
                nc.scalar.activation(
                    wt, wt, AF.Abs,
                    accum_out=wsums[:, half * WHALF + j:half * WHALF + j + 1])
            col = vecs.tile([P, 1], F32, tag="wcol")
            nc.vector.tensor_reduce(
                col, wsums[:, half * WHALF:(half + 1) * WHALF],
                axis=AX.X, op=ALU.add)
            pssc = ps_v.tile([1, 1], F32, tag="psv")
            pe(nc.tensor.matmul(pssc, col, ones[:, 0:1], start=True,
                                stop=True))
            cc_sb = vecs.tile([1, 1], F32, tag="cc_sb")
            nc.scalar.copy(cc_sb, pssc)
            cc_in = cc1_in if half == 0 else cc2_in
            cc_out = cc1_out if half == 0 else cc2_out
            nc.sync.dma_start(cc_in, cc_sb)
            nc.gpsimd.collective_compute(
                "AllGather", ALU.bypass,
                replica_groups=[list(range(NCORES))],
                ins=[cc_in.opt()], outs=[cc_out.opt()])

        def mw_materialize(half):
            cc_out = cc1_out if half == 0 else cc2_out
            cc8 = vecs.tile([NCORES, 1], F32, tag="cc8")
            nc.sync.dma_start(cc8, cc_out)
            pss = ps_v.tile([1, 1], F32, tag="psv")
            pe(nc.tensor.matmul(pss, ones[0:NCORES, 0:1], cc8, start=True,
                                stop=True))
            sc = vecs.tile([1, 1], F32, tag="sc")
            nc.scalar.copy(sc, pss)
            psbc = ps_v.tile([P, 1], F32, tag="psv")
            pe(nc.tensor.matmul(psbc, ones[0:1, :], sc, start=True,
                                stop=True))
            nc.scalar.mul(mws[:, half:half + 1], psbc,
                          1.0 / (INNER * DIM * 127.0))

        def finalize_batch(stv_all, M_all, WID, r_all, c_all, nchunk):
            # stv_all [P,TT,nchunk,6]; all other args [P,TT] column packs
            mvs = vecs.tile([P, TT, 2], F32, tag="bn_mvs")
            for tt in range(TT):
                nc.vector.bn_aggr(mvs[:, tt, :], stv_all[:, tt, :, :])
            msq = vecs.tile([P, TT], F32, tag="msqs")
            nc.vector.tensor_tensor(msq, mvs[:, :, 0], mvs[:, :, 0],
                                    op=ALU.mult)
            nc.vector.tensor_tensor(msq, msq, mvs[:, :, 1], op=ALU.add)
            nc.vector.tensor_scalar_add(msq, msq, EPS_RMS)
            y = vecs.tile([P, TT], F32, tag="sq_ys")
            nc.scalar.sqrt(y, msq)
            d_ = vecs.tile([P, TT], F32, tag="sq_ds")
            nc.vector.reciprocal(d_, y)
            nc.vector.tensor_tensor(d_, msq, d_, op=ALU.mult)
            nc.vector.tensor_tensor(y, y, d_, op=ALU.add)
            nc.vector.tensor_scalar_mul(y, y, 0.5 * (float(WID) ** 0.5))
            a = vecs.tile([P, TT], F32, tag="as_")
            nc.vector.reciprocal(a, y)
            nc.vector.tensor_tensor(c_all, a, M_all, op=ALU.mult)
            nc.vector.tensor_scalar_max(c_all, c_all, EPS_Q)
            r = vecs.tile([P, TT], F32, tag="rs_")
            nc.vector.reciprocal(r, c_all)
            nc.vector.tensor_tensor(r, r, a, op=ALU.mult)
            nc.vector.tensor_scalar_mul(r_all, r, 127.0)

        def col_to_row(col, row_slice):
            pst = ps_v.tile([1, P], F32, tag="psv")
            pe(nc.tensor.transpose(pst, col, identf))
            nc.scalar.copy(row_slice, pst)

        # ===== A: |w| shard sums; AG1 issued before x-phase ============
        wshard_sums(0)

        # ===== B pass 1: x stats + batched finalize + quantize =========
        q1T = qTp.tile([P, KD, TOK], BF16, tag="qT", name="q1T")
        for tt in range(TT):
            for cc in range(4):
                xt = stagx.tile([P, 512], F32, tag="stagx")
                nc.sync.dma_start(xt, x_d[tt * P:(tt + 1) * P,
                                          cc * 512:(cc + 1) * 512])
                nc.vector.bn_stats(stvx[:, tt, cc, :], xt)
                mx = vecs.tile([P, 1], F32, tag="mx")
                nc.vector.tensor_reduce(mx, xt, axis=AX.X, op=ALU.max,
                                        apply_absolute_value=True)
                M1 = M1s[:, tt:tt + 1]
                if cc == 0:
                    nc.vector.tensor_copy(out=M1, in_=mx)
                else:
                    nc.vector.tensor_tensor(M1, M1, mx, op=ALU.max)
        finalize_batch(stvx, M1s, DIM, r1s, c1s, 4)
        wshard_sums(1)
        for tt in range(TT):
            for cc in range(4):
                xt = stagx.tile([P, 512], F32, tag="stagx")
                nc.sync.dma_start(xt, x_d[tt * P:(tt + 1) * P,
                                          cc * 512:(cc + 1) * 512])
                xq = stagx.tile([P, 512], F32, tag="stagx")
                nc.vector.tensor_scalar(xq, xt, r1s[:, tt:tt + 1], C_RND,
                                        op0=ALU.mult, op1=ALU.add)
                q1 = qp.tile([P, 512], BF16, tag="q1")
                nc.scalar.activation(q1, xq, AF.Copy, bias=-C_RND)
                pst = ps_t.tile([P, 512], BF16, tag="pst")
                for j in range(4):
                    pe(nc.tensor.transpose(pst[:, j * P:(j + 1) * P],
                                           q1[:, j * P:(j + 1) * P], identb))
                nc.vector.tensor_copy(
                    out=q1T[:, 4 * cc:4 * (cc + 1), tt * P:(tt + 1) * P],
                    in_=pst.rearrange("p (a b) -> p a b", b=P))

        # ===== B pass 2: mean|w1|-dependent scales =====================
        mw_materialize(0)
        rb1s = vecs.tile([P, TT], F32, tag="rb1s")
        nc.vector.tensor_scalar(beta1s, c1s, mws[:, 0:1], None, op0=ALU.mult)
        nc.vector.reciprocal(rb1s, beta1s)
        for tt in range(TT):
            col_to_row(rb1s[:, tt:tt + 1], rb1T[0:1, tt * P:(tt + 1) * P])

        # ===== C: L1 slabs =============================================
        for e in range(NE1):
            ws = wring.tile([P, KD, SL1], BF16, tag="w", name=f"ws1_{e}")
            for dc in range(KD):
                wt = stag.tile([P, SL1], F32, tag="stag")
                nc.sync.dma_start(
                    wt, w1t_d[dc * P:(dc + 1) * P, e * SL1:(e + 1) * SL1])
                nc.scalar.sign(ws[:, dc, :], wt)
            bc = bch.tile([1, SL1], BF16, tag="bc")
            nc.sync.dma_start(bc, b1_d[0:1, e * SL1:(e + 1) * SL1])
            for tt in range(TT):
                pg = ps_g.tile([P, SL1], F32, tag="psg")
                pe(nc.tensor.matmul(pg, rb1T[0:1, tt * P:(tt + 1) * P], bc,
                                    start=True, stop=False))
                for dc in range(KD):
                    pe(nc.tensor.matmul(pg, q1T[:, dc, tt * P:(tt + 1) * P],
                                        ws[:, dc, :], start=False,
                                        stop=(dc == KD - 1)))
                hc = hcp.tile([P, SL1], BF16, tag="h")
                nc.scalar.activation(hc, pg, AF.Gelu,
                                     scale=beta1s[:, tt:tt + 1])
                nc.vector.bn_stats(stv2[:, tt, e, :], hc)
                mx = vecs.tile([P, 1], F32, tag="mx")
                nc.vector.tensor_reduce(mx, hc, axis=AX.X, op=ALU.max,
                                        apply_absolute_value=True)
                m2 = m2s[:, tt:tt + 1]
                if e == 0:
                    nc.vector.tensor_copy(out=m2, in_=mx)
                else:
                    nc.vector.tensor_tensor(m2, m2, mx, op=ALU.max)
                nc.sync.dma_start(
                    h_dram[tt * P:(tt + 1) * P, e * SL1:(e + 1) * SL1], hc)

        # ===== D: batched L2 scales + q2T rebuild ======================
        mw_materialize(1)
        c2s = vecs.tile([P, TT], F32, tag="c2s")
        finalize_batch(stv2, m2s, INNER, r2s, c2s, NE1)
        beta2s = vecs.tile([P, TT], F32, tag="beta2s")
        nc.vector.tensor_scalar(beta2s, c2s, mws[:, 1:2], None, op0=ALU.mult)
        rb2s = vecs.tile([P, TT], F32, tag="rb2s")
        nc.vector.reciprocal(rb2s, beta2s)
        for tt in range(TT):
            col_to_row(rb2s[:, tt:tt + 1], rb2row[0:1, tt * P:(tt + 1) * P])
            col_to_row(beta2s[:, tt:tt + 1],
                       beta2row[0:1, tt * P:(tt + 1) * P])
        bb0 = bb.tile([P, 512], F32, tag="bb0")
        bb1 = bb.tile([P, 512], F32, tag="bb1")
        nc.gpsimd.partition_broadcast(bb0, beta2row[0:1, 0:512])
        nc.gpsimd.partition_broadcast(bb1, beta2row[0:1, 512:1024])
        bbs = [bb0, bb1]

        q2T = qTp.tile([P, KI, TOK], BF16, tag="qT", name="q2T")
        for tt in range(TT):
            for ic in range(INNER // 512):
                hr = hcp.tile([P, 512], BF16, tag="h")
                nc.sync.dma_start(
                    hr, h_dram[tt * P:(tt + 1) * P, ic * 512:(ic + 1) * 512])
                hq = stagx.tile([P, 512], F32, tag="stagx")
                nc.vector.tensor_scalar(hq, hr, r2s[:, tt:tt + 1], C_RND,
                                        op0=ALU.mult, op1=ALU.add)
                q2c = hcp.tile([P, 512], BF16, tag="q2c")
                nc.scalar.activation(q2c, hq, AF.Copy, bias=-C_RND)
                pst = ps_t.tile([P, 512], BF16, tag="pst")
                for j in range(4):
                    pe(nc.tensor.transpose(pst[:, j * P:(j + 1) * P],
                                           q2c[:, j * P:(j + 1) * P], identb))
                nc.vector.tensor_copy(
                    out=q2T[:, 4 * ic:4 * (ic + 1), tt * P:(tt + 1) * P],
                    in_=pst.rearrange("p (a b) -> p a b", b=P))

        # ===== E: L2 o-bands ===========================================
        for b in range(NB2):
            ws2 = wring.tile([P, KI, BO], BF16, tag="w", name=f"ws2_{b}")
            for g in range(16):
                wt = stag.tile([P, 4, BO], F32, tag="stag")
                nc.sync.dma_start(
                    wt,
                    w2t_d[g * 512:(g + 1) * 512,
                          b * BO:(b + 1) * BO].rearrange(
                              "(k p) o -> p k o", p=P))
                nc.scalar.sign(ws2[:, 4 * g:4 * (g + 1), :], wt)
            bc2 = bch.tile([1, BO], BF16, tag="bc")
            nc.sync.dma_start(bc2, b2_d[0:1, b * BO:(b + 1) * BO])
            pbs = [ps_g.tile([P, 512], F32, tag="psg", name=f"pb{b}_{t}")
                   for t in range(2)]
            for tg in range(2):
                pe(nc.tensor.matmul(pbs[tg], bc2,
                                    rb2row[0:1, tg * 512:(tg + 1) * 512],
                                    start=True, stop=False))
            for kc in range(KI):
                for tg in range(2):
                    pe(nc.tensor.matmul(pbs[tg], ws2[:, kc, :],
                                        q2T[:, kc, tg * 512:(tg + 1) * 512],
                                        start=False, stop=(kc == KI - 1)))
            for tg in range(2):
                ob = outp.tile([P, 512], BF16, tag="ob")
                nc.vector.tensor_tensor(ob, pbs[tg], bbs[tg], op=ALU.mult)
                nc.sync.dma_start(
                    out_d[b * BO:(b + 1) * BO, tg * 512:(tg + 1) * 512], ob)

    nc.compile()
    return nc


@functools.lru_cache(maxsize=1)
def _get_nc():
    return build()


def kernel(x, w1, b1, w2, b2, _trace=False):
    nc = _get_nc()
    xf = np.ascontiguousarray(x.reshape(B * S, DIM), dtype=np.float32)
    w1 = np.asarray(w1, dtype=np.float32)
    w2 = np.asarray(w2, dtype=np.float32)
    w1f = w1.reshape(-1)
    w2f = w2.reshape(-1)
    shard = w1f.size // NCORES
    common = {
        "w1t": np.ascontiguousarray(w1.T),
        "w2t": np.ascontiguousarray(w2.T),
        "b1": np.asarray(b1, dtype=np.float32).reshape(1, INNER).astype(
            ml_dtypes.bfloat16),
        "b2": np.asarray(b2, dtype=np.float32).reshape(1, OUT).astype(
            ml_dtypes.bfloat16),
        "identf": np.eye(P, dtype=np.float32),
        "identb": np.eye(P, dtype=np.float32).astype(ml_dtypes.bfloat16),
        "ones": np.ones((P, P), dtype=np.float32),
    }
    in_maps = []
    for c in range(NCORES):
        wsh = np.concatenate([
            w1f[c * shard:(c + 1) * shard].reshape(P, -1),
            w2f[c * shard:(c + 1) * shard].reshape(P, -1)], axis=1)
        in_maps.append({
            "x": xf[c * TOK:(c + 1) * TOK],
            "wsh": np.ascontiguousarray(wsh),
            **common,
        })
    res = run_bass_kernel_spmd(nc, in_maps, core_ids=list(range(NCORES)),
                               trace=_trace)
    out = np.concatenate(
        [res.results[c]["out"].astype(np.float32).T for c in range(NCORES)],
        axis=0)
    out = out.reshape(B, S, DIM)
    if _trace:
        return out, res
    return out


# revision 37
# speedup vs baseline: 1.3415x; 1.0656x over previous
"""BitFeedForward (BitNet-style FFN) Trainium2 kernel — 8-core data parallel.

kernel(**inputs) takes the FULL unsharded inputs of
nn_BitFeedForward_25280177504455:
    x  [4, 2048, 2048] f32, w1 [8192, 2048], b1 [8192],
    w2 [2048, 8192], b2 [2048]
and returns the full [4, 2048, 2048] f32 output.

Sharding: data-parallel over tokens (1024 tokens/core).  The host ships
weights pre-transposed (w1.T / w2.T, a pure layout change) so each core
streams them with large contiguous DMA descriptors, converts to ternary
sign form on the fly (sign -> bf16 in SBUF, no DRAM spill), and uses
them directly as matmul operands.  weight_quant's global mean|w| is
reduced from per-core shards with two tiny 8-core AllGathers (8 bytes),
issued early so their ~50us control-plane latency hides behind the
x-quantization phase.

Per-core flow:
  A. |w| partial sums over this core's weight shard (scalar engine),
     AllGather the two partials across cores.
  B. x stats -> batched scale finalize ([P,8] vector ops, one sqrt) ->
     quantize + PE-transpose -> q1T resident in SBUF.  The mean|w|-
     dependent beta1/rb1 math is emitted after this block so no engine
     queue stalls on the collectives.
  C. L1: stream w1T in 512-col slabs, sign to bf16, matmul
     h[t,i] = gelu(beta1*(q1T.T @ w1s + rb1 x b1)); per-token bn_stats/
     absmax accumulate; h spilled bf16.
  D. batched r2/beta2 finalize; rebuild q2 from h, PE-transpose -> q2T
     (aliases q1T's SBUF).
  E. L2: stream w2T in 128-row o-bands, sign, matmul both 512-token
     groups interleaved per stationary chunk:
     out[o,t] = beta2[t]*(w2s.T @ q2T + b2 x rb2), written bf16 [o,t];
     host transposes and upcasts.
"""
import functools

import numpy as np
import ml_dtypes

from contextlib import ExitStack

import concourse.bacc as bacc
import concourse.tile as tile
from concourse import mybir
from concourse.bass_utils import run_bass_kernel_spmd

F32 = mybir.dt.float32
BF16 = mybir.dt.bfloat16

EPS_RMS = 1e-6
EPS_Q = 1e-5
# v + C lands in [2^23, 2^24) where fp32 spacing is 1.0 -> RNE integer round
C_RND = float(1.5 * 2.0**23)
P = 128
AX = mybir.AxisListType
ALU = mybir.AluOpType
AF = mybir.ActivationFunctionType

NCORES = 8
B, S, DIM = 4, 2048, 2048
INNER = 8192
OUT = DIM
TOK = B * S // NCORES   # 1024 tokens per core
TT = TOK // P           # 8 token tiles
KD = DIM // P           # 16 contraction chunks for L1
KI = INNER // P         # 64 contraction chunks for L2
NE1 = 16                # L1 slabs (512 inner cols each)
SL1 = INNER // NE1      # 512
NB2 = 16                # L2 o-bands (128 out cols each)
BO = OUT // NB2         # 128
WSHE = 2 * INNER * DIM // NCORES  # weight-shard elements per core (4.19M)
WHALF = WSHE // (2 * P * 1024)    # 1024-chunks per w half (16)


def build():
    from concourse.tile_rust import add_dep_helper

    nc = bacc.Bacc("TRN2", enable_partition_id=False, num_devices=NCORES)

    x_d = nc.dram_tensor("x", [TOK, DIM], F32, kind="ExternalInput")
    w1t_d = nc.dram_tensor("w1t", [DIM, INNER], F32, kind="ExternalInput")
    # w2 host-tiled: [g, band, p, k*BO] so each band-tile load has 2KB
    # contiguous partition lines (g: 16 i-blocks of 512, band: 16 o-bands)
    w2t_d = nc.dram_tensor("w2t", [16 * NB2 * P, 4 * BO], F32,
                           kind="ExternalInput")
    b1_d = nc.dram_tensor("b1", [1, INNER], BF16, kind="ExternalInput")
    b2_d = nc.dram_tensor("b2", [1, OUT], BF16, kind="ExternalInput")
    wsh_d = nc.dram_tensor("wsh", [P, WSHE // P], F32, kind="ExternalInput")
    idf_d = nc.dram_tensor("identf", [P, P], F32, kind="ExternalInput")
    idb_d = nc.dram_tensor("identb", [P, P], BF16, kind="ExternalInput")
    ones_d = nc.dram_tensor("ones", [P, P], F32, kind="ExternalInput")
    out_d = nc.dram_tensor("out", [OUT, TOK], BF16, kind="ExternalOutput")

    with ExitStack() as ctx:
        tc = ctx.enter_context(tile.TileContext(nc))
        pool = lambda name, bufs, space="SBUF": ctx.enter_context(
            tc.tile_pool(name=name, bufs=bufs, space=space))

        consts = pool("consts", 1)
        stag = pool("stag", 2)        # f32 staging: w1T/w2T slab tiles
        stagx = pool("stagx", 3)      # f32 staging: x / h-requant tiles
        wshp = pool("wshp", 2)        # f32 staging: wsh tiles
        wring = pool("wring", 2)      # bf16 sign-weight slabs
        qTp = pool("qTp", 1)          # q1T then q2T (aliased)
        qp = pool("qp", 1)            # q1 bf16 chunks
        hcp = pool("hcp", 3)          # h / q2 bf16 tiles
        outp = pool("outp", 2)        # bf16 output drains
        vecs = pool("vecs", 4)
        bch = pool("bch", 1)
        bb = pool("bb", 1)
        pers = pool("pers", 1)
        dram = pool("dram", 1, "DRAM")
        ps_g = pool("ps_g", 4, "PSUM")
        ps_t = pool("ps_t", 2, "PSUM")
        ps_v = pool("ps_v", 2, "PSUM")

        identf = consts.tile([P, P], F32)
        identb = consts.tile([P, P], BF16)
        ones = consts.tile([P, P], F32)
        nc.sync.dma_start(identf, idf_d[:, :])
        nc.sync.dma_start(identb, idb_d[:, :])
        nc.sync.dma_start(ones, ones_d[:, :])

        h_dram = dram.tile([TOK, INNER], BF16)
        cc1_in = dram.tile([1, 1], F32)
        cc1_out = dram.tile([NCORES, 1], F32, addr_space="Shared")
        cc2_in = dram.tile([1, 1], F32)
        cc2_out = dram.tile([NCORES, 1], F32, addr_space="Shared")

        state = {"pe": None}

        def pe(instr):
            if state["pe"] is not None:
                add_dep_helper(instr.ins, state["pe"].ins, sync=False,
                               reason="pe chain")
            state["pe"] = instr
            return instr

        # ---- persistent scalars/vectors ----
        wsums = pers.tile([P, 2 * WHALF], F32, tag="wsums")
        stvx = pers.tile([P, TT, 4, 6], F32, tag="stvx")
        M1s = pers.tile([P, TT], F32, tag="M1s")
        r1s = pers.tile([P, TT], F32, tag="r1s")
        c1s = pers.tile([P, TT], F32, tag="c1s")
        beta1s = pers.tile([P, TT], F32, tag="beta1s")
        r2s = pers.tile([P, TT], F32, tag="r2s")
        m2s = pers.tile([P, TT], F32, tag="m2s")
        rb1T = pers.tile([1, TOK], BF16, tag="rb1T")
        rb2row = pers.tile([1, TOK], BF16, tag="rb2row")
        beta2row = pers.tile([1, TOK], BF16, tag="beta2row")
        mws = pers.tile([P, 2], F32, tag="mws")
        stv2 = pers.tile([P, TT, NE1, 6], F32, tag="stv2")

        def wshard_sums(half):
            for j in range(WHALF):
                wt = wshp.tile([P, 1024], F32, tag="wsh")
                nc.scalar.dma_start(
                    wt, wsh_d[:, (half * WHALF + j) * 1024:
                              (half * WHALF + j + 1) * 1024])
                nc.scalar.activation(
                    wt, wt, AF.Abs,
                    accum_out=wsums[:, half * WHALF + j:half * WHALF + j + 1])
            col = vecs.tile([P, 1], F32, tag="wcol")
            nc.vector.tensor_reduce(
                col, wsums[:, half * WHALF:(half + 1) * WHALF],
                axis=AX.X, op=ALU.add)
            pssc = ps_v.tile([1, 1], F32, tag="psv")
            pe(nc.tensor.matmul(pssc, col, ones[:, 0:1], start=True,
                                stop=True))
            cc_sb = vecs.tile([1, 1], F32, tag="cc_sb")
            nc.scalar.copy(cc_sb, pssc)
            cc_in = cc1_in if half == 0 else cc2_in
            cc_out = cc1_out if half == 0 else cc2_out
            nc.sync.dma_start(cc_in, cc_sb)
            nc.gpsimd.collective_compute(
                "AllGather", ALU.bypass,
                replica_groups=[list(range(NCORES))],
                ins=[cc_in.opt()], outs=[cc_out.opt()])

        def mw_materialize(half):
            cc_out = cc1_out if half == 0 else cc2_out
            cc8 = vecs.tile([NCORES, 1], F32, tag="cc8")
            nc.sync.dma_start(cc8, cc_out)
            pss = ps_v.tile([1, 1], F32, tag="psv")
            pe(nc.tensor.matmul(pss, ones[0:NCORES, 0:1], cc8, start=True,
                                stop=True))
            sc = vecs.tile([1, 1], F32, tag="sc")
            nc.scalar.copy(sc, pss)
            psbc = ps_v.tile([P, 1], F32, tag="psv")
            pe(nc.tensor.matmul(psbc, ones[0:1, :], sc, start=True,
                                stop=True))
            nc.scalar.mul(mws[:, half:half + 1], psbc,
                          1.0 / (INNER * DIM * 127.0))

        def finalize_batch(stv_all, M_all, WID, r_all, c_all, nchunk):
            # stv_all [P,TT,nchunk,6]; all other args [P,TT] column packs
            mvs = vecs.tile([P, TT, 2], F32, tag="bn_mvs")
            for tt in range(TT):
                nc.vector.bn_aggr(mvs[:, tt, :], stv_all[:, tt, :, :])
            msq = vecs.tile([P, TT], F32, tag="msqs")
            nc.vector.tensor_tensor(msq, mvs[:, :, 0], mvs[:, :, 0],
                                    op=ALU.mult)
            nc.vector.tensor_tensor(msq, msq, mvs[:, :, 1], op=ALU.add)
            nc.vector.tensor_scalar_add(msq, msq, EPS_RMS)
            y = vecs.tile([P, TT], F32, tag="sq_ys")
            nc.scalar.sqrt(y, msq)
            d_ = vecs.tile([P, TT], F32, tag="sq_ds")
            nc.vector.reciprocal(d_, y)
            nc.vector.tensor_tensor(d_, msq, d_, op=ALU.mult)
            nc.vector.tensor_tensor(y, y, d_, op=ALU.add)
            nc.vector.tensor_scalar_mul(y, y, 0.5 * (float(WID) ** 0.5))
            a = vecs.tile([P, TT], F32, tag="as_")
            nc.vector.reciprocal(a, y)
            nc.vector.tensor_tensor(c_all, a, M_all, op=ALU.mult)
            nc.vector.tensor_scalar_max(c_all, c_all, EPS_Q)
            r = vecs.tile([P, TT], F32, tag="rs_")
            nc.vector.reciprocal(r, c_all)
            nc.vector.tensor_tensor(r, r, a, op=ALU.mult)
            nc.vector.tensor_scalar_mul(r_all, r, 127.0)

        def col_to_row(col, row_slice):
            pst = ps_v.tile([1, P], F32, tag="psv")
            pe(nc.tensor.transpose(pst, col, identf))
            nc.scalar.copy(row_slice, pst)

        # ===== A: |w| shard sums; AG1 issued before x-phase ============
        wshard_sums(0)

        # ===== B pass 1: x stats + batched finalize + quantize =========
        q1T = qTp.tile([P, KD, TOK], BF16, tag="qT", name="q1T")
        for tt in range(TT):
            for cc in range(4):
                xt = stagx.tile([P, 512], F32, tag="stagx")
                nc.sync.dma_start(xt, x_d[tt * P:(tt + 1) * P,
                                          cc * 512:(cc + 1) * 512])
                nc.vector.bn_stats(stvx[:, tt, cc, :], xt)
                mx = vecs.tile([P, 1], F32, tag="mx")
                nc.vector.tensor_reduce(mx, xt, axis=AX.X, op=ALU.max,
                                        apply_absolute_value=True)
                M1 = M1s[:, tt:tt + 1]
                if cc == 0:
                    nc.vector.tensor_copy(out=M1, in_=mx)
                else:
                    nc.vector.tensor_tensor(M1, M1, mx, op=ALU.max)
        finalize_batch(stvx, M1s, DIM, r1s, c1s, 4)
        for tt in range(TT):
            for cc in range(4):
                xt = stagx.tile([P, 512], F32, tag="stagx")
                nc.sync.dma_start(xt, x_d[tt * P:(tt + 1) * P,
                                          cc * 512:(cc + 1) * 512])
                xq = stagx.tile([P, 512], F32, tag="stagx")
                nc.vector.tensor_scalar(xq, xt, r1s[:, tt:tt + 1], C_RND,
                                        op0=ALU.mult, op1=ALU.add)
                q1 = qp.tile([P, 512], BF16, tag="q1")
                nc.scalar.activation(q1, xq, AF.Copy, bias=-C_RND)
                pst = ps_t.tile([P, 512], BF16, tag="pst")
                for j in range(4):
                    pe(nc.tensor.transpose(pst[:, j * P:(j + 1) * P],
                                           q1[:, j * P:(j + 1) * P], identb))
                nc.vector.tensor_copy(
                    out=q1T[:, 4 * cc:4 * (cc + 1), tt * P:(tt + 1) * P],
                    in_=pst.rearrange("p (a b) -> p a b", b=P))

        # ===== B pass 2: mean|w1|-dependent scales =====================
        mw_materialize(0)
        rb1s = vecs.tile([P, TT], F32, tag="rb1s")
        nc.vector.tensor_scalar(beta1s, c1s, mws[:, 0:1], None, op0=ALU.mult)
        nc.vector.reciprocal(rb1s, beta1s)
        for tt in range(TT):
            col_to_row(rb1s[:, tt:tt + 1], rb1T[0:1, tt * P:(tt + 1) * P])
        wshard_sums(1)

        # ===== C: L1 slabs =============================================
        for e in range(NE1):
            ws = wring.tile([P, KD, SL1], BF16, tag="w", name=f"ws1_{e}")
            for dc in range(KD):
                wt = stag.tile([P, SL1], F32, tag="stag")
                nc.sync.dma_start(
                    wt, w1t_d[dc * P:(dc + 1) * P, e * SL1:(e + 1) * SL1])
                nc.scalar.sign(ws[:, dc, :], wt)
            bc = bch.tile([1, SL1], BF16, tag="bc")
            nc.sync.dma_start(bc, b1_d[0:1, e * SL1:(e + 1) * SL1])
            for tt in range(TT):
                pg = ps_g.tile([P, SL1], F32, tag="psg")
                pe(nc.tensor.matmul(pg, rb1T[0:1, tt * P:(tt + 1) * P], bc,
                                    start=True, stop=False))
                for dc in range(KD):
                    pe(nc.tensor.matmul(pg, q1T[:, dc, tt * P:(tt + 1) * P],
                                        ws[:, dc, :], start=False,
                                        stop=(dc == KD - 1)))
                hc = hcp.tile([P, SL1], BF16, tag="h")
                nc.scalar.activation(hc, pg, AF.Gelu,
                                     scale=beta1s[:, tt:tt + 1])
                nc.vector.bn_stats(stv2[:, tt, e, :], hc)
                mx = vecs.tile([P, 1], F32, tag="mx")
                nc.vector.tensor_reduce(mx, hc, axis=AX.X, op=ALU.max,
                                        apply_absolute_value=True)
                m2 = m2s[:, tt:tt + 1]
                if e == 0:
                    nc.vector.tensor_copy(out=m2, in_=mx)
                else:
                    nc.vector.tensor_tensor(m2, m2, mx, op=ALU.max)
                nc.sync.dma_start(
                    h_dram[tt * P:(tt + 1) * P, e * SL1:(e + 1) * SL1], hc)

        # ===== D: batched L2 scales + q2T rebuild ======================
        mw_materialize(1)
        c2s = vecs.tile([P, TT], F32, tag="c2s")
        finalize_batch(stv2, m2s, INNER, r2s, c2s, NE1)
        beta2s = vecs.tile([P, TT], F32, tag="beta2s")
        nc.vector.tensor_scalar(beta2s, c2s, mws[:, 1:2], None, op0=ALU.mult)
        rb2s = vecs.tile([P, TT], F32, tag="rb2s")
        nc.vector.reciprocal(rb2s, beta2s)
        for tt in range(TT):
            col_to_row(rb2s[:, tt:tt + 1], rb2row[0:1, tt * P:(tt + 1) * P])
            col_to_row(beta2s[:, tt:tt + 1],
                       beta2row[0:1, tt * P:(tt + 1) * P])
        bb0 = bb.tile([P, 512], BF16, tag="bb0")
        bb1 = bb.tile([P, 512], BF16, tag="bb1")
        nc.gpsimd.partition_broadcast(bb0, beta2row[0:1, 0:512])
        nc.gpsimd.partition_broadcast(bb1, beta2row[0:1, 512:1024])
        bbs = [bb0, bb1]

        q2T = qTp.tile([P, KI, TOK], BF16, tag="qT", name="q2T")
        for tt in range(TT):
            for ic in range(INNER // 512):
                hr = hcp.tile([P, 512], BF16, tag="h")
                nc.sync.dma_start(
                    hr, h_dram[tt * P:(tt + 1) * P, ic * 512:(ic + 1) * 512])
                hq = stagx.tile([P, 512], F32, tag="stagx")
                nc.vector.tensor_scalar(hq, hr, r2s[:, tt:tt + 1], C_RND,
                                        op0=ALU.mult, op1=ALU.add)
                q2c = hcp.tile([P, 512], BF16, tag="q2c")
                nc.scalar.activation(q2c, hq, AF.Copy, bias=-C_RND)
                pst = ps_t.tile([P, 512], BF16, tag="pst")
                for j in range(4):
                    pe(nc.tensor.transpose(pst[:, j * P:(j + 1) * P],
                                           q2c[:, j * P:(j + 1) * P], identb))
                nc.vector.tensor_copy(
                    out=q2T[:, 4 * ic:4 * (ic + 1), tt * P:(tt + 1) * P],
                    in_=pst.rearrange("p (a b) -> p a b", b=P))

        # ===== E: L2 o-bands ===========================================
        for b in range(NB2):
            ws2 = wring.tile([P, KI, BO], BF16, tag="w", name=f"ws2_{b}")
            for g in range(16):
                wt = stag.tile([P, 4, BO], F32, tag="stag")
                nc.sync.dma_start(
                    wt,
                    w2t_d[(g * NB2 + b) * P:(g * NB2 + b + 1) * P,
                          :].rearrange("p (k o) -> p k o", o=BO))
                nc.scalar.sign(ws2[:, 4 * g:4 * (g + 1), :], wt)
            bc2 = bch.tile([1, BO], BF16, tag="bc")
            nc.sync.dma_start(bc2, b2_d[0:1, b * BO:(b + 1) * BO])
            pbs = [ps_g.tile([P, 512], F32, tag="psg", name=f"pb{b}_{t}")
                   for t in range(2)]
            for tg in range(2):
                pe(nc.tensor.matmul(pbs[tg], bc2,
                                    rb2row[0:1, tg * 512:(tg + 1) * 512],
                                    start=True, stop=False))
            for kc in range(KI):
                for tg in range(2):
                    pe(nc.tensor.matmul(pbs[tg], ws2[:, kc, :],
                                        q2T[:, kc, tg * 512:(tg + 1) * 512],
                                        start=False, stop=(kc == KI - 1)))
            for tg in range(2):
                ob = outp.tile([P, 512], BF16, tag="ob")
                nc.vector.tensor_tensor(ob, pbs[tg], bbs[tg], op=ALU.mult)
                nc.sync.dma_start(
                    out_d[b * BO:(b + 1) * BO, tg * 512:(tg + 1) * 512], ob)

    nc.compile()
    return nc


@functools.lru_cache(maxsize=1)
def _get_nc():
    return build()


def kernel(x, w1, b1, w2, b2, _trace=False):
    nc = _get_nc()
    xf = np.ascontiguousarray(x.reshape(B * S, DIM), dtype=np.float32)
    w1 = np.asarray(w1, dtype=np.float32)
    w2 = np.asarray(w2, dtype=np.float32)
    w1f = w1.reshape(-1)
    w2f = w2.reshape(-1)
    shard = w1f.size // NCORES
    common = {
        "w1t": np.ascontiguousarray(w1.T),
        "w2t": np.ascontiguousarray(
            w2.T.reshape(16, 4, P, NB2, BO).transpose(0, 3, 2, 1, 4)
            .reshape(16 * NB2 * P, 4 * BO)),
        "b1": np.asarray(b1, dtype=np.float32).reshape(1, INNER).astype(
            ml_dtypes.bfloat16),
        "b2": np.asarray(b2, dtype=np.float32).reshape(1, OUT).astype(
            ml_dtypes.bfloat16),
        "identf": np.eye(P, dtype=np.float32),
        "identb": np.eye(P, dtype=np.float32).astype(ml_dtypes.bfloat16),
        "ones": np.ones((P, P), dtype=np.float32),
    }
    in_maps = []
    for c in range(NCORES):
        wsh = np.concatenate([
            w1f[c * shard:(c + 1) * shard].reshape(P, -1),
            w2f[c * shard:(c + 1) * shard].reshape(P, -1)], axis=1)
        in_maps.append({
            "x": xf[c * TOK:(c + 1) * TOK],
            "wsh": np.ascontiguousarray(wsh),
            **common,
        })
    res = run_bass_kernel_spmd(nc, in_maps, core_ids=list(range(NCORES)),
                               trace=_trace)
    out = np.concatenate(
        [res.results[c]["out"].astype(np.float32).T for c in range(NCORES)],
        axis=0)
    out = out.reshape(B, S, DIM)
    if _trace:
        return out, res
    return out
